# revision 14
# baseline (speedup 1.0000x reference)
"""Trainium2 Bass kernel for grouped full attention with dynamic relative
position bias (8 heads, 400 tokens/group, dim 256, batch 128).

Strategy: pure data parallel over the 128 (batch*group) rows — 16 per core.
The tiny position-bias MLP runs on host (it only depends on the small weight
inputs); the device kernel computes, per batch group:
  qkT = (Wqk^T x^T), v = x Wv          (fp32r matmuls)
  S^T = K Q^T (scaled)                  (row-packed per head pair)
  E = exp(S^T) * exp(rpb^T)             (ACT exp + DVE/GPSIMD bf16 mul)
  U^T = V^T E, sums = 1^T E             (col-tiled 4 heads per matmul)
  out = (U / sums) @ Wp                 (fp32r)
"""

import math

import numpy as np
import ml_dtypes

import concourse.bass as bass
import concourse.mybir as mybir
import concourse.tile as tile
from concourse import bacc
from concourse.bass import ts
from concourse.bass_utils import run_bass_kernel_spmd

# ---- problem constants (hardcoded per contract) ----
T, V = 16, 25
N = T * V              # 400 tokens per group
DIM = 256
HEADS = 8
HEAD_DIM = 32
SCALE = HEAD_DIM ** -0.5
LN_EPS = 1e-5
B_ = 128
NCORES = 8
BPC = B_ // NCORES     # 16 batch groups per core
NPAD = 512             # token dim padded to 4 partition chunks
MC = 4                 # m chunks (128,128,128,16)

F32 = mybir.dt.float32
F32R = mybir.dt.float32r
BF16 = mybir.dt.bfloat16

_CACHE = {}


def _pos_mlp_host(posproj_w, posproj_b, ln1_g, ln1_b, p1_w, p1_b,
                  ln2_g, ln2_b, p2_w, p2_b, ln3_g, ln3_b, p3_w, p3_b):
    """Replicates the reference dynamic position bias MLP in numpy fp32."""
    bh = np.arange(1 - T, T, dtype=np.float32)
    bw = np.arange(1 - V, V, dtype=np.float32)
    grid = np.stack(np.meshgrid(bh, bw, indexing="ij"))       # [2, 2T-1, 2V-1]
    biases = grid.reshape(2, -1).T.astype(np.float32)         # [(2T-1)(2V-1), 2]

    def layernorm(x, g, b):
        mu = x.mean(axis=-1, keepdims=True)
        var = x.var(axis=-1, keepdims=True)
        return (x - mu) / np.sqrt(var + LN_EPS) * g + b

    pos = biases @ posproj_w + posproj_b
    pos = np.maximum(layernorm(pos, ln1_g, ln1_b), 0.0) @ p1_w + p1_b
    pos = np.maximum(layernorm(pos, ln2_g, ln2_b), 0.0) @ p2_w + p2_b
    pos = np.maximum(layernorm(pos, ln3_g, ln3_b), 0.0) @ p3_w + p3_b
    return pos.astype(np.float32)                             # [(2T-1)(2V-1), HEADS]


def _rel_idx_host():
    coords = np.stack(np.meshgrid(np.arange(T), np.arange(V), indexing="ij"))
    cf = coords.reshape(2, -1)                                 # [2, N]
    rel = (cf[:, :, None] - cf[:, None, :]).transpose(1, 2, 0)  # [N, N, 2]
    rel[:, :, 0] += T - 1
    rel[:, :, 1] += V - 1
    rel[:, :, 0] *= 2 * V - 1
    return rel.sum(-1).astype(np.int32)                        # [N, N]


def _emit(ctx, tc, d, vb_nonzero, pb_nonzero, bpc):
    nc = tc.nc

    const = ctx.enter_context(tc.tile_pool(name="const", bufs=1))
    xt_pool = ctx.enter_context(tc.tile_pool(name="xt", bufs=3))
    qk_pool = ctx.enter_context(tc.tile_pool(name="qk", bufs=2))
    v_pool = ctx.enter_context(tc.tile_pool(name="v", bufs=2))
    e_pool = ctx.enter_context(tc.tile_pool(name="e", bufs=2))
    rs_pool = ctx.enter_context(tc.tile_pool(name="rs", bufs=2))
    un_pool = ctx.enter_context(tc.tile_pool(name="un", bufs=2))
    ps_sc = ctx.enter_context(tc.tile_pool(name="ps_sc", bufs=2, space="PSUM"))
    ps_av = ctx.enter_context(tc.tile_pool(name="ps_av", bufs=1, space="PSUM"))
    ps_u = ctx.enter_context(tc.tile_pool(name="ps_u", bufs=2, space="PSUM"))

    # ---- resident constants ----
    w_qk = const.tile([128, 2, 512], F32R)
    nc.sync.dma_start(w_qk[:], d["w_qk"][:])
    w_v = const.tile([128, 2, 256], F32R)
    nc.sync.dma_start(w_v[:], d["w_v"][:])
    w_p = const.tile([128, 2, 256], F32R)
    nc.sync.dma_start(w_p[:], d["w_p"][:])
    bqk = const.tile([128, 4], F32)
    nc.sync.dma_start(bqk[:], d["bqk"][:])
    rb = const.tile([128, HEADS, MC, N], BF16)
    nc.sync.dma_start(rb[:], d["rbias"][:])
    ones = const.tile([128, 32], BF16)
    nc.vector.memset(ones[:], 1.0)
    if vb_nonzero:
        vb = const.tile([128, 256], F32)
        nc.sync.dma_start(
            vb[:],
            bass.AP(tensor=d["bv"].tensor, offset=d["bv"].offset,
                    ap=[[0, 128]] + d["bv"].ap),
        )
    if pb_nonzero:
        pb = const.tile([128, 256], F32)
        nc.sync.dma_start(
            pb[:],
            bass.AP(tensor=d["bp"].tensor, offset=d["bp"].offset,
                    ap=[[0, 128]] + d["bp"].ap),
        )
    out_pool = ctx.enter_context(tc.tile_pool(name="outp", bufs=3))

    for b in range(bpc):
        xt = xt_pool.tile([128, 2, N], F32R)
        nc.sync.dma_start(xt[:], d["xt"][b])

        # ---- qkT = Wqk^T @ x^T : [512 c', 400 j] as 4 tiles ----
        qkT = qk_pool.tile([128, 4, N], F32R)
        for t in range(4):
            ps = ps_u.tile([128, N], F32)
            for cc in range(2):
                nc.tensor.matmul(
                    ps[:], w_qk[:, cc, ts(t, 128)], xt[:, cc, :],
                    start=(cc == 0), stop=(cc == 1),
                )
            nc.vector.tensor_scalar(
                out=qkT[:, t, :], in0=ps[:], scalar1=bqk[:, t:t + 1],
                scalar2=None, op0=mybir.AluOpType.add,
            )

        # ---- v = x @ Wv : [400 tok, 256 c'] natural, bf16 ----
        v = v_pool.tile([128, MC, 256], BF16)
        for nt in range(4):
            m = 128 if nt < 3 else N - 3 * 128
            ps = ps_u.tile([128, 256], F32)
            for cc in range(2):
                nc.tensor.matmul(
                    ps[0:m, :],
                    xt[:, cc, nt * 128:nt * 128 + m],
                    w_v[:, cc, :],
                    start=(cc == 0), stop=(cc == 1),
                )
            if vb_nonzero:
                nc.vector.tensor_tensor(
                    out=v[0:m, nt, :], in0=ps[0:m, :], in1=vb[0:m, :],
                    op=mybir.AluOpType.add)
            else:
                nc.vector.tensor_copy(out=v[0:m, nt, :], in_=ps[0:m, :])

        # ---- attention ----
        E = e_pool.tile([128, HEADS, MC, N], BF16)
        recS = rs_pool.tile([128, 2, N], F32)
        unT = un_pool.tile([128, 2, N], F32R)
        for q in range(2):
            # scores + exp for the quad (4 heads), in head-pair units
            for mc in range(MC):
                m = 128 if mc < 3 else N - 3 * 128
                for pair in range(2):
                    ps = ps_sc.tile([128, 2, 512], F32)
                    for i in range(2):
                        h = 4 * q + 2 * pair + i
                        rbase = 32 * (h % 4)
                        ktile = 2 + h // 4
                        qtile = h // 4
                        nc.tensor.matmul(
                            ps[0:m, i, 0:N],
                            qkT[rbase:rbase + 32, ktile,
                                    mc * 128:mc * 128 + m],
                            qkT[rbase:rbase + 32, qtile, :],
                            start=True, stop=True,
                            tile_position=(rbase, 0),
                        )
                    h0 = 4 * q + 2 * pair
                    nc.scalar.activation(
                        out=E[0:m, h0:h0 + 2, mc, :],
                        in_=ps[0:m, :, 0:N],
                        func=mybir.ActivationFunctionType.Exp,
                    )
                    # bias multiply: DVE, except one mc slice on gpsimd
                    eng = nc.gpsimd if mc == 1 else nc.vector
                    eng.tensor_tensor(
                        out=E[0:m, h0:h0 + 2, mc, :],
                        in0=E[0:m, h0:h0 + 2, mc, :],
                        in1=rb[0:m, h0:h0 + 2, mc, :],
                        op=mybir.AluOpType.mult,
                    )
            # AV + sums, accumulated over mc. Each psum bank holds 4
            # col-tiled accumulation groups (32-partition ranges); the
            # sim's group-region check aliases partition offsets, but the
            # per-partition zero semantics are correct — skip the check.
            av = ps_av.tile([128, 2, 512], F32)
            for mc in range(MC):
                k = 128 if mc < 3 else N - 3 * 128
                for bank, lhs_of in (
                    (0, lambda h: v[0:k, mc, 32 * h:32 * h + 32]),
                    (1, lambda h: ones[0:k, :]),
                ):
                    for h4 in range(4):
                        h = 4 * q + h4
                        nc.tensor.matmul(
                            av[32 * h4:32 * h4 + 32, bank, 0:N],
                            lhs_of(h),
                            E[0:k, h, mc, :],
                            start=(mc == 0), stop=(mc == 3),
                            tile_position=(0, 32 * h4),
                            skip_group_check=True,
                        )
            nc.vector.reciprocal_approx_fast(
                out=recS[:, q, :], in_=av[:, 1, 0:N])
            nc.vector.tensor_tensor(
                out=unT[:, q, :], in0=av[:, 0, 0:N], in1=recS[:, q, :],
                op=mybir.AluOpType.mult,
            )

        # ---- proj: out[ntile, 256] = Un^T.T @ Wp ----
        for nt in range(4):
            m = 128 if nt < 3 else N - 3 * 128
            ps = ps_u.tile([128, 256], F32)
            for cc in range(2):
                nc.tensor.matmul(
                    ps[0:m, :],
                    unT[:, cc, nt * 128:nt * 128 + m],
                    w_p[:, cc, :],
                    start=(cc == 0), stop=(cc == 1),
                )
            o = out_pool.tile([128, 256], F32)
            if pb_nonzero:
                nc.vector.tensor_tensor(
                    out=o[0:m, :], in0=ps[0:m, :], in1=pb[0:m, :],
                    op=mybir.AluOpType.add)
            else:
                nc.vector.tensor_copy(out=o[0:m, :], in_=ps[0:m, :])
            nc.sync.dma_start(d["out"][b, nt * 128:nt * 128 + m], o[0:m, :])


def _build(vb_nonzero, pb_nonzero, bpc=BPC):
    nc = bacc.Bacc("TRN2", target_bir_lowering=False, debug=False,
                   num_devices=NCORES)
    d = {}
    d["xt"] = nc.dram_tensor("xt", [bpc, 128, 2, N], F32R,
                             kind="ExternalInput").ap()
    d["w_qk"] = nc.dram_tensor("w_qk", [128, 2, 512], F32R,
                               kind="ExternalInput").ap()
    d["w_v"] = nc.dram_tensor("w_v", [128, 2, 256], F32R,
                              kind="ExternalInput").ap()
    d["w_p"] = nc.dram_tensor("w_p", [128, 2, 256], F32R,
                              kind="ExternalInput").ap()
    d["bqk"] = nc.dram_tensor("bqk", [128, 4], F32,
                              kind="ExternalInput").ap()
    d["rbias"] = nc.dram_tensor("rbias", [128, HEADS, MC, N], BF16,
                                kind="ExternalInput").ap()
    if vb_nonzero:
        d["bv"] = nc.dram_tensor("bv", [256], F32, kind="ExternalInput").ap()
    if pb_nonzero:
        d["bp"] = nc.dram_tensor("bp", [256], F32, kind="ExternalInput").ap()
    d["out"] = nc.dram_tensor("out", [bpc, N, DIM], F32,
                              kind="ExternalOutput").ap()

    from contextlib import ExitStack

    with tile.TileContext(nc) as tc:
        with ExitStack() as ctx:
            _emit(ctx, tc, d, vb_nonzero, pb_nonzero, bpc)
    nc.compile()
    return nc, d


def _prep_host(inputs):
    x = np.ascontiguousarray(np.asarray(inputs["x"], dtype=np.float32))
    qkv_w = np.asarray(inputs["qkv_w"], dtype=np.float32)
    qkv_b = np.asarray(inputs["qkv_b"], dtype=np.float32)
    proj_w = np.asarray(inputs["proj_w"], dtype=np.float32)
    proj_b = np.asarray(inputs["proj_b"], dtype=np.float32)

    pos = _pos_mlp_host(
        *[np.asarray(inputs[k], dtype=np.float32) for k in (
            "posproj_w", "posproj_b", "ln1_g", "ln1_b", "p1_w", "p1_b",
            "ln2_g", "ln2_b", "p2_w", "p2_b", "ln3_g", "ln3_b",
            "p3_w", "p3_b")])
    rel = _rel_idx_host()
    rpb = pos[rel.reshape(-1)].reshape(N, N, HEADS)       # [n, m, h]
    rbiasT = np.exp(rpb.transpose(2, 1, 0))               # [h, m, n]
    rpad = np.ones((HEADS, NPAD, N), np.float32)
    rpad[:, :N, :] = rbiasT
    rbias_dev = np.ascontiguousarray(
        rpad.reshape(HEADS, MC, 128, N).transpose(2, 0, 1, 3)
    ).astype(ml_dtypes.bfloat16)                          # [128, h, mc, n]

    w_qk = qkv_w[:, :512].copy()
    w_qk[:, :256] *= SCALE
    w_qk_dev = np.ascontiguousarray(
        w_qk.reshape(2, 128, 512).transpose(1, 0, 2))
    w_v_dev = np.ascontiguousarray(
        qkv_w[:, 512:].reshape(2, 128, 256).transpose(1, 0, 2))
    w_p_dev = np.ascontiguousarray(
        proj_w.reshape(2, 128, 256).transpose(1, 0, 2))
    b_qk = qkv_b[:512].copy()
    b_qk[:256] *= SCALE
    bqk_dev = np.ascontiguousarray(b_qk.reshape(4, 128).T)

    b_v = qkv_b[512:]
    vb_nonzero = bool(np.any(b_v != 0))
    pb_nonzero = bool(np.any(proj_b != 0))

    # x^T per core: [BPC, 128 (c within chunk), 2 (chunk), 400]
    xt_all = np.ascontiguousarray(
        x.transpose(0, 2, 1).reshape(B_, 2, 128, N).transpose(0, 2, 1, 3))

    common = {
        "w_qk": w_qk_dev, "w_v": w_v_dev, "w_p": w_p_dev,
        "bqk": bqk_dev, "rbias": rbias_dev,
    }
    if vb_nonzero:
        common["bv"] = np.ascontiguousarray(b_v)
    if pb_nonzero:
        common["bp"] = np.ascontiguousarray(proj_b)
    in_maps = []
    for c in range(NCORES):
        m = dict(common)
        m["xt"] = np.ascontiguousarray(xt_all[c * BPC:(c + 1) * BPC])
        in_maps.append(m)
    return in_maps, vb_nonzero, pb_nonzero


def kernel(**inputs) -> np.ndarray:
    in_maps, vb_nonzero, pb_nonzero = _prep_host(inputs)
    key = (vb_nonzero, pb_nonzero)
    if key not in _CACHE:
        _CACHE[key] = _build(vb_nonzero, pb_nonzero)
    nc, _ = _CACHE[key]
    res = run_bass_kernel_spmd(nc, in_maps, core_ids=list(range(NCORES)))
    out = np.concatenate([res.results[c]["out"] for c in range(NCORES)], axis=0)
    return out.astype(np.float32)


def run_traced(**inputs):
    """Like kernel() but with NTFF tracing; returns (out, BassKernelResults)."""
    in_maps, vb_nonzero, pb_nonzero = _prep_host(inputs)
    key = (vb_nonzero, pb_nonzero)
    if key not in _CACHE:
        _CACHE[key] = _build(vb_nonzero, pb_nonzero)
    nc, _ = _CACHE[key]
    res = run_bass_kernel_spmd(nc, in_maps, core_ids=list(range(NCORES)),
                               trace=True)
    out = np.concatenate([res.results[c]["out"] for c in range(NCORES)], axis=0)
    return out.astype(np.float32), res


# revision 15
# speedup vs baseline: 1.0791x; 1.0791x over previous
"""Trainium2 Bass kernel for grouped full attention with dynamic relative
position bias (8 heads, 400 tokens/group, dim 256, batch 128).

Strategy: pure data parallel over the 128 (batch*group) rows — 16 per core.
The tiny position-bias MLP runs on host (it only depends on the small weight
inputs); the device kernel computes, per batch group:
  qkT = (Wqk^T x^T), v = x Wv          (fp32r matmuls)
  S^T = K Q^T (scaled)                  (row-packed per head pair)
  E = exp(S^T) * exp(rpb^T)             (ACT exp + DVE/GPSIMD bf16 mul)
  U^T = V^T E, sums = 1^T E             (col-tiled 4 heads per matmul)
  out = (U / sums) @ Wp                 (fp32r)
"""

import math

import numpy as np
import ml_dtypes

import concourse.bass as bass
import concourse.mybir as mybir
import concourse.tile as tile
from concourse import bacc
from concourse.bass import ts
from concourse.bass_utils import run_bass_kernel_spmd

# ---- problem constants (hardcoded per contract) ----
T, V = 16, 25
N = T * V              # 400 tokens per group
DIM = 256
HEADS = 8
HEAD_DIM = 32
SCALE = HEAD_DIM ** -0.5
LN_EPS = 1e-5
B_ = 128
NCORES = 8
BPC = B_ // NCORES     # 16 batch groups per core
NPAD = 512             # token dim padded to 4 partition chunks
MC = 4                 # m chunks (128,128,128,16)

F32 = mybir.dt.float32
F32R = mybir.dt.float32r  # unused after bf16 switch
BF16 = mybir.dt.bfloat16

_CACHE = {}


def _pos_mlp_host(posproj_w, posproj_b, ln1_g, ln1_b, p1_w, p1_b,
                  ln2_g, ln2_b, p2_w, p2_b, ln3_g, ln3_b, p3_w, p3_b):
    """Replicates the reference dynamic position bias MLP in numpy fp32."""
    bh = np.arange(1 - T, T, dtype=np.float32)
    bw = np.arange(1 - V, V, dtype=np.float32)
    grid = np.stack(np.meshgrid(bh, bw, indexing="ij"))       # [2, 2T-1, 2V-1]
    biases = grid.reshape(2, -1).T.astype(np.float32)         # [(2T-1)(2V-1), 2]

    def layernorm(x, g, b):
        mu = x.mean(axis=-1, keepdims=True)
        var = x.var(axis=-1, keepdims=True)
        return (x - mu) / np.sqrt(var + LN_EPS) * g + b

    pos = biases @ posproj_w + posproj_b
    pos = np.maximum(layernorm(pos, ln1_g, ln1_b), 0.0) @ p1_w + p1_b
    pos = np.maximum(layernorm(pos, ln2_g, ln2_b), 0.0) @ p2_w + p2_b
    pos = np.maximum(layernorm(pos, ln3_g, ln3_b), 0.0) @ p3_w + p3_b
    return pos.astype(np.float32)                             # [(2T-1)(2V-1), HEADS]


def _rel_idx_host():
    coords = np.stack(np.meshgrid(np.arange(T), np.arange(V), indexing="ij"))
    cf = coords.reshape(2, -1)                                 # [2, N]
    rel = (cf[:, :, None] - cf[:, None, :]).transpose(1, 2, 0)  # [N, N, 2]
    rel[:, :, 0] += T - 1
    rel[:, :, 1] += V - 1
    rel[:, :, 0] *= 2 * V - 1
    return rel.sum(-1).astype(np.int32)                        # [N, N]


def _emit(ctx, tc, d, vb_nonzero, pb_nonzero, bpc):
    nc = tc.nc

    const = ctx.enter_context(tc.tile_pool(name="const", bufs=1))
    xt_pool = ctx.enter_context(tc.tile_pool(name="xt", bufs=3))
    qk_pool = ctx.enter_context(tc.tile_pool(name="qk", bufs=2))
    v_pool = ctx.enter_context(tc.tile_pool(name="v", bufs=2))
    e_pool = ctx.enter_context(tc.tile_pool(name="e", bufs=2))
    rs_pool = ctx.enter_context(tc.tile_pool(name="rs", bufs=2))
    un_pool = ctx.enter_context(tc.tile_pool(name="un", bufs=2))
    ps_sc = ctx.enter_context(tc.tile_pool(name="ps_sc", bufs=2, space="PSUM"))
    ps_av = ctx.enter_context(tc.tile_pool(name="ps_av", bufs=1, space="PSUM"))
    ps_u = ctx.enter_context(tc.tile_pool(name="ps_u", bufs=2, space="PSUM"))

    # ---- resident constants ----
    w_qk = const.tile([128, 2, 512], BF16)
    nc.sync.dma_start(w_qk[:], d["w_qk"][:])
    w_v = const.tile([128, 2, 256], BF16)
    nc.sync.dma_start(w_v[:], d["w_v"][:])
    w_p = const.tile([128, 2, 256], BF16)
    nc.sync.dma_start(w_p[:], d["w_p"][:])
    bqk = const.tile([128, 4], F32)
    nc.sync.dma_start(bqk[:], d["bqk"][:])
    rb = const.tile([128, HEADS, MC, N], BF16)
    nc.sync.dma_start(rb[:], d["rbias"][:])
    ones = const.tile([128, 32], BF16)
    nc.vector.memset(ones[:], 1.0)
    if vb_nonzero:
        vb = const.tile([128, 256], F32)
        nc.sync.dma_start(
            vb[:],
            bass.AP(tensor=d["bv"].tensor, offset=d["bv"].offset,
                    ap=[[0, 128]] + d["bv"].ap),
        )
    if pb_nonzero:
        pb = const.tile([128, 256], F32)
        nc.sync.dma_start(
            pb[:],
            bass.AP(tensor=d["bp"].tensor, offset=d["bp"].offset,
                    ap=[[0, 128]] + d["bp"].ap),
        )
    out_pool = ctx.enter_context(tc.tile_pool(name="outp", bufs=3))

    for b in range(bpc):
        xt = xt_pool.tile([128, 2, N], BF16)
        nc.sync.dma_start(xt[:], d["xt"][b])

        # ---- qkT = Wqk^T @ x^T : [512 c', 400 j] as 4 tiles ----
        qkT = qk_pool.tile([128, 4, N], BF16)
        for t in range(4):
            ps = ps_u.tile([128, N], F32)
            for cc in range(2):
                nc.tensor.matmul(
                    ps[:], w_qk[:, cc, ts(t, 128)], xt[:, cc, :],
                    start=(cc == 0), stop=(cc == 1),
                )
            nc.vector.tensor_scalar(
                out=qkT[:, t, :], in0=ps[:], scalar1=bqk[:, t:t + 1],
                scalar2=None, op0=mybir.AluOpType.add,
            )

        # ---- v = x @ Wv : [400 tok, 256 c'] natural, bf16 ----
        v = v_pool.tile([128, MC, 256], BF16)
        for nt in range(4):
            m = 128 if nt < 3 else N - 3 * 128
            ps = ps_u.tile([128, 256], F32)
            for cc in range(2):
                nc.tensor.matmul(
                    ps[0:m, :],
                    xt[:, cc, nt * 128:nt * 128 + m],
                    w_v[:, cc, :],
                    start=(cc == 0), stop=(cc == 1),
                )
            if vb_nonzero:
                nc.vector.tensor_tensor(
                    out=v[0:m, nt, :], in0=ps[0:m, :], in1=vb[0:m, :],
                    op=mybir.AluOpType.add)
            else:
                nc.vector.tensor_copy(out=v[0:m, nt, :], in_=ps[0:m, :])

        # ---- attention ----
        E = e_pool.tile([128, HEADS, MC, N], BF16)
        recS = rs_pool.tile([128, 2, N], F32)
        unT = un_pool.tile([128, 2, N], BF16)
        for q in range(2):
            # scores + exp for the quad (4 heads), in head-pair units
            for mc in range(MC):
                m = 128 if mc < 3 else N - 3 * 128
                for pair in range(2):
                    ps = ps_sc.tile([128, 2, 512], F32)
                    for i in range(2):
                        h = 4 * q + 2 * pair + i
                        rbase = 32 * (h % 4)
                        ktile = 2 + h // 4
                        qtile = h // 4
                        nc.tensor.matmul(
                            ps[0:m, i, 0:N],
                            qkT[rbase:rbase + 32, ktile,
                                    mc * 128:mc * 128 + m],
                            qkT[rbase:rbase + 32, qtile, :],
                            start=True, stop=True,
                            tile_position=(rbase, 0),
                        )
                    h0 = 4 * q + 2 * pair
                    nc.scalar.activation(
                        out=E[0:m, h0:h0 + 2, mc, :],
                        in_=ps[0:m, :, 0:N],
                        func=mybir.ActivationFunctionType.Exp,
                    )
                    # bias multiply: DVE, except one mc slice on gpsimd
                    eng = nc.gpsimd if mc == 1 else nc.vector
                    eng.tensor_tensor(
                        out=E[0:m, h0:h0 + 2, mc, :],
                        in0=E[0:m, h0:h0 + 2, mc, :],
                        in1=rb[0:m, h0:h0 + 2, mc, :],
                        op=mybir.AluOpType.mult,
                    )
            # AV + sums, accumulated over mc. Each psum bank holds 4
            # col-tiled accumulation groups (32-partition ranges); the
            # sim's group-region check aliases partition offsets, but the
            # per-partition zero semantics are correct — skip the check.
            av = ps_av.tile([128, 2, 512], F32)
            for mc in range(MC):
                k = 128 if mc < 3 else N - 3 * 128
                for bank, lhs_of in (
                    (0, lambda h: v[0:k, mc, 32 * h:32 * h + 32]),
                    (1, lambda h: ones[0:k, :]),
                ):
                    for h4 in range(4):
                        h = 4 * q + h4
                        nc.tensor.matmul(
                            av[32 * h4:32 * h4 + 32, bank, 0:N],
                            lhs_of(h),
                            E[0:k, h, mc, :],
                            start=(mc == 0), stop=(mc == 3),
                            tile_position=(0, 32 * h4),
                            skip_group_check=True,
                        )
            nc.vector.reciprocal_approx_fast(
                out=recS[:, q, :], in_=av[:, 1, 0:N])
            nc.vector.tensor_tensor(
                out=unT[:, q, :], in0=av[:, 0, 0:N], in1=recS[:, q, :],
                op=mybir.AluOpType.mult,
            )

        # ---- proj: out[ntile, 256] = Un^T.T @ Wp ----
        for nt in range(4):
            m = 128 if nt < 3 else N - 3 * 128
            ps = ps_u.tile([128, 256], F32)
            for cc in range(2):
                nc.tensor.matmul(
                    ps[0:m, :],
                    unT[:, cc, nt * 128:nt * 128 + m],
                    w_p[:, cc, :],
                    start=(cc == 0), stop=(cc == 1),
                )
            o = out_pool.tile([128, 256], F32)
            if pb_nonzero:
                nc.vector.tensor_tensor(
                    out=o[0:m, :], in0=ps[0:m, :], in1=pb[0:m, :],
                    op=mybir.AluOpType.add)
            else:
                nc.vector.tensor_copy(out=o[0:m, :], in_=ps[0:m, :])
            nc.sync.dma_start(d["out"][b, nt * 128:nt * 128 + m], o[0:m, :])


def _build(vb_nonzero, pb_nonzero, bpc=BPC):
    nc = bacc.Bacc("TRN2", target_bir_lowering=False, debug=False,
                   num_devices=NCORES)
    d = {}
    d["xt"] = nc.dram_tensor("xt", [bpc, 128, 2, N], BF16,
                             kind="ExternalInput").ap()
    d["w_qk"] = nc.dram_tensor("w_qk", [128, 2, 512], BF16,
                               kind="ExternalInput").ap()
    d["w_v"] = nc.dram_tensor("w_v", [128, 2, 256], BF16,
                              kind="ExternalInput").ap()
    d["w_p"] = nc.dram_tensor("w_p", [128, 2, 256], BF16,
                              kind="ExternalInput").ap()
    d["bqk"] = nc.dram_tensor("bqk", [128, 4], F32,
                              kind="ExternalInput").ap()
    d["rbias"] = nc.dram_tensor("rbias", [128, HEADS, MC, N], BF16,
                                kind="ExternalInput").ap()
    if vb_nonzero:
        d["bv"] = nc.dram_tensor("bv", [256], F32, kind="ExternalInput").ap()
    if pb_nonzero:
        d["bp"] = nc.dram_tensor("bp", [256], F32, kind="ExternalInput").ap()
    d["out"] = nc.dram_tensor("out", [bpc, N, DIM], F32,
                              kind="ExternalOutput").ap()

    from contextlib import ExitStack

    with tile.TileContext(nc) as tc:
        with ExitStack() as ctx:
            _emit(ctx, tc, d, vb_nonzero, pb_nonzero, bpc)
    nc.compile()
    return nc, d


def _prep_host(inputs):
    x = np.ascontiguousarray(np.asarray(inputs["x"], dtype=np.float32))
    qkv_w = np.asarray(inputs["qkv_w"], dtype=np.float32)
    qkv_b = np.asarray(inputs["qkv_b"], dtype=np.float32)
    proj_w = np.asarray(inputs["proj_w"], dtype=np.float32)
    proj_b = np.asarray(inputs["proj_b"], dtype=np.float32)

    pos = _pos_mlp_host(
        *[np.asarray(inputs[k], dtype=np.float32) for k in (
            "posproj_w", "posproj_b", "ln1_g", "ln1_b", "p1_w", "p1_b",
            "ln2_g", "ln2_b", "p2_w", "p2_b", "ln3_g", "ln3_b",
            "p3_w", "p3_b")])
    rel = _rel_idx_host()
    rpb = pos[rel.reshape(-1)].reshape(N, N, HEADS)       # [n, m, h]
    rbiasT = np.exp(rpb.transpose(2, 1, 0))               # [h, m, n]
    rpad = np.ones((HEADS, NPAD, N), np.float32)
    rpad[:, :N, :] = rbiasT
    rbias_dev = np.ascontiguousarray(
        rpad.reshape(HEADS, MC, 128, N).transpose(2, 0, 1, 3)
    ).astype(ml_dtypes.bfloat16)                          # [128, h, mc, n]

    w_qk = qkv_w[:, :512].copy()
    w_qk[:, :256] *= SCALE
    w_qk_dev = np.ascontiguousarray(
        w_qk.reshape(2, 128, 512).transpose(1, 0, 2)).astype(ml_dtypes.bfloat16)
    w_v_dev = np.ascontiguousarray(
        qkv_w[:, 512:].reshape(2, 128, 256).transpose(1, 0, 2)).astype(
        ml_dtypes.bfloat16)
    w_p_dev = np.ascontiguousarray(
        proj_w.reshape(2, 128, 256).transpose(1, 0, 2)).astype(
        ml_dtypes.bfloat16)
    b_qk = qkv_b[:512].copy()
    b_qk[:256] *= SCALE
    bqk_dev = np.ascontiguousarray(b_qk.reshape(4, 128).T)

    b_v = qkv_b[512:]
    vb_nonzero = bool(np.any(b_v != 0))
    pb_nonzero = bool(np.any(proj_b != 0))

    # x^T per core: [BPC, 128 (c within chunk), 2 (chunk), 400]
    xt_all = np.ascontiguousarray(
        x.transpose(0, 2, 1).reshape(B_, 2, 128, N).transpose(0, 2, 1, 3)
    ).astype(ml_dtypes.bfloat16)

    common = {
        "w_qk": w_qk_dev, "w_v": w_v_dev, "w_p": w_p_dev,
        "bqk": bqk_dev, "rbias": rbias_dev,
    }
    if vb_nonzero:
        common["bv"] = np.ascontiguousarray(b_v)
    if pb_nonzero:
        common["bp"] = np.ascontiguousarray(proj_b)
    in_maps = []
    for c in range(NCORES):
        m = dict(common)
        m["xt"] = np.ascontiguousarray(xt_all[c * BPC:(c + 1) * BPC])
        in_maps.append(m)
    return in_maps, vb_nonzero, pb_nonzero


def kernel(**inputs) -> np.ndarray:
    in_maps, vb_nonzero, pb_nonzero = _prep_host(inputs)
    key = (vb_nonzero, pb_nonzero)
    if key not in _CACHE:
        _CACHE[key] = _build(vb_nonzero, pb_nonzero)
    nc, _ = _CACHE[key]
    res = run_bass_kernel_spmd(nc, in_maps, core_ids=list(range(NCORES)))
    out = np.concatenate([res.results[c]["out"] for c in range(NCORES)], axis=0)
    return out.astype(np.float32)


def run_traced(**inputs):
    """Like kernel() but with NTFF tracing; returns (out, BassKernelResults)."""
    in_maps, vb_nonzero, pb_nonzero = _prep_host(inputs)
    key = (vb_nonzero, pb_nonzero)
    if key not in _CACHE:
        _CACHE[key] = _build(vb_nonzero, pb_nonzero)
    nc, _ = _CACHE[key]
    res = run_bass_kernel_spmd(nc, in_maps, core_ids=list(range(NCORES)),
                               trace=True)
    out = np.concatenate([res.results[c]["out"] for c in range(NCORES)], axis=0)
    return out.astype(np.float32), res
